# revision 10
# baseline (speedup 1.0000x reference)
"""Causal multi-head attention (B=4, T=2048, D=1024, H=16) on 8 TRN2 NeuronCores.

Sharding: tensor-parallel over heads. Each core owns 2 heads (a contiguous
128-column block of each of W_q / W_k / W_v and a 128-row block of W_out).
Every core consumes the full (transposed) activation matrix xT and produces a
partial output [B*T, D]; the host sums the 8 partials (the "all-reduce").

Per-core device pipeline (all matmuls in float32r — full PE rate, ~1e-4 rel):
  phase A (per batch b): QT2[128,T] = Wq2h.T @ xT_b, KT2 likewise,
           V2[T,128] = xT_b.T @ Wv2h  (stored as Vaug tiles [128,(64|1)x2]
           with a ones column appended per head).
  phase B (per b, per 512-wide query chunk qc): for each 128-wide key tile kt:
           ST_h[k,q] = KT2_h.T @ QT2_h  (scores, transposed layout),
           causal mask via additive -1e30 tile on the diagonal blocks,
           expST = exp(ST/8) on ACT (straight-through softmax, no max-sub),
           OT_h[65,512] += Vaug_h.T @ expST  (row 64 accumulates the softmax
           denominator via the ones column).
  phase C (per b, qc): recip = 1/OT[64,:], broadcast across partitions with a
           rank-1 PE matmul, OTn = OT[0:64]*recip (both heads -> [128,512]),
           out[t,:] partial = OTn.T @ W_out2h, DMA to DRAM.
"""

import sys
import os

if "/opt/trn_rl_repo" not in sys.path:
    sys.path.insert(0, "/opt/trn_rl_repo")

import numpy as np

B, T, D, H, HD = 4, 2048, 1024, 16, 64
NCORES = 8
HPC = H // NCORES          # heads per core = 2
WBLK = HPC * HD            # 128: per-core head-block width
QC = 512                   # query chunk (matmul moving dim)
NQC = T // QC              # 4
KT = 128                   # key tile
NKT = T // KT              # 16
DC = 128                   # contraction chunk of D
NDC = D // DC              # 8

TRACE = False              # test.py sets kernel.TRACE = True for profiling
LAST_EXEC_NS = None
LAST_RESULTS = None

_MAX_WAITS = 1


def _make_tc_class():
    """TileContext patched for this container's walrus build, which rejects
    instructions carrying more than one sync-wait command (CTRL Drain,
    S3_LW ldweights, ...). Excess waits are hoisted onto freshly inserted
    same-engine NOPs placed immediately before the instruction (engine
    queues are in-order, so semantics are preserved)."""
    import concourse.tile as tile
    import concourse.mybir as mybir
    from concourse.vector_clock import VectorClock, ScopedClock

    class TC(tile.TileContext):
        def _drain_and_barrier(self, tick_clock, wait_clock):
            g = tick_clock.global_clock
            n = len(g)
            for proc in range(n):
                t = g[proc]
                if t > 0:
                    nop = self.nc.sync.nop(nofuse=True)
                    vc = VectorClock([0] * n)
                    vc.require_at_least(proc, t)
                    wait_clock.add_sem_waits(nop.ins, ScopedClock({None: vc}))
            self.nc.sync.drain()
            self.nc.all_engine_barrier()
            popped = self.nc._tile_sem_poison_stack.pop()
            assert popped is self._sem_poison
            self.nc.clear_and_free_semaphores(list(self.sems.allocated().values()))
            self.nc.all_engine_barrier()

        def _lower_ordered_insts(self, ordered):
            for bb_name in list(ordered.keys()):
                insts = ordered[bb_name]
                new_insts = []
                for inst in insts:
                    si = inst.sync_info
                    ow = list(si.on_wait) if si is not None and si.on_wait else []
                    if len(ow) > _MAX_WAITS:
                        keep = ow[:_MAX_WAITS]
                        extra = ow[_MAX_WAITS:]
                        for w in extra:
                            nop = mybir.InstNoOp(
                                name=f"WSPL-{self.nc.next_id()}", ins=[], outs=[]
                            )
                            nop.engine = inst.engine
                            nop.bass_nofuse = True
                            nop.sync_info = mybir.SyncInfo(on_wait=[w], on_update=[])
                            new_insts.append(nop)
                        inst.sync_info = mybir.SyncInfo(
                            on_wait=keep,
                            on_update=list(si.on_update) if si.on_update else [],
                        )
                    new_insts.append(inst)
                ordered[bb_name] = new_insts
            return super()._lower_ordered_insts(ordered)

    return TC


def _install_ntff_hook():
    """Provide antenv.axon_hooks (absent from the container's antenv stub) so
    run_bass_kernel_spmd(trace=True) can capture NTFF profiles."""
    import types
    import antenv

    if "antenv.axon_hooks" in sys.modules:
        return
    mod = types.ModuleType("antenv.axon_hooks")
    mod._hook = None
    mod.set_axon_ntff_profile_hook = lambda h: setattr(mod, "_hook", h)
    mod.get_axon_ntff_profile_hook = lambda: mod._hook
    sys.modules["antenv.axon_hooks"] = mod
    antenv.axon_hooks = mod
    try:
        from trn_agent_boot.trn_boot import _ntff_profile_via_ctypes

        hook = _ntff_profile_via_ctypes("/opt/axon/libaxon_pjrt.so")
        if hook is not None:
            mod.set_axon_ntff_profile_hook(hook)
    except Exception as e:  # profiling is best-effort
        print("ntff hook install failed:", e)


def _build_program():
    import concourse.bass as bass
    from concourse import mybir

    TC = _make_tc_class()
    f32 = mybir.dt.float32
    f32r = mybir.dt.float32r
    Exp = mybir.ActivationFunctionType.Exp
    BT = B * T

    nc = bass.Bass("TRN2", target_bir_lowering=False, debug=False, num_devices=NCORES)
    xt_d = nc.dram_tensor("xt", [D, BT], f32, kind="ExternalInput")
    wq_d = nc.dram_tensor("wq", [D, WBLK], f32, kind="ExternalInput")
    wk_d = nc.dram_tensor("wk", [D, WBLK], f32, kind="ExternalInput")
    wv_d = nc.dram_tensor("wv", [D, WBLK], f32, kind="ExternalInput")
    wo_d = nc.dram_tensor("wo", [WBLK, D], f32, kind="ExternalInput")
    ones_d = nc.dram_tensor("ones", [128, 64], f32, kind="ExternalInput")
    out_d = nc.dram_tensor("out", [BT, D], f32, kind="ExternalOutput")

    with TC(nc, num_cores=NCORES) as tc:
        from contextlib import ExitStack

        with ExitStack() as ctx:
            constp = ctx.enter_context(tc.tile_pool(name="constp", bufs=1))
            wp = ctx.enter_context(tc.tile_pool(name="wp", bufs=1))
            xtp = ctx.enter_context(tc.tile_pool(name="xtp", bufs=10))
            qkp = ctx.enter_context(tc.tile_pool(name="qkp", bufs=2))
            vap = ctx.enter_context(tc.tile_pool(name="vap", bufs=2))
            expp = ctx.enter_context(tc.tile_pool(name="expp", bufs=3))
            otnp = ctx.enter_context(tc.tile_pool(name="otnp", bufs=2))
            outsbp = ctx.enter_context(tc.tile_pool(name="outsbp", bufs=3))
            rcpp = ctx.enter_context(tc.tile_pool(name="rcpp", bufs=2))
            bcp = ctx.enter_context(tc.tile_pool(name="bcp", bufs=2))
            # PSUM: st(3) + mix(3) + ot0(1) + ot1(1) = 8 banks
            stp = ctx.enter_context(tc.tile_pool(name="stp", bufs=3, space="PSUM"))
            mixp = ctx.enter_context(tc.tile_pool(name="mixp", bufs=3, space="PSUM"))
            otp = ctx.enter_context(tc.tile_pool(name="otp", bufs=1, space="PSUM"))

            # ---- one-time constants / weights ----
            ones64 = constp.tile([1, 64], f32r)
            nc.gpsimd.dma_start(ones64[:], ones_d[0:1, 0:64])
            maskadd = []
            for j in range(QC // KT):  # 4 diagonal offsets
                m = constp.tile([128, QC], f32, name=f"maskadd{j}")
                nc.gpsimd.memset(m[:], 0.0)
                # keep (add 0) where q - k >= 0: q = qc0 + y, k = qc0 + 128j + x
                nc.gpsimd.affine_select(
                    out=m[:],
                    in_=m[:],
                    compare_op=mybir.AluOpType.is_ge,
                    fill=-1e30,
                    base=-KT * j,
                    pattern=[[1, QC]],
                    channel_multiplier=-1,
                )
                maskadd.append(m)

            wq_t = wp.tile([DC, NDC, WBLK], f32r)
            nc.gpsimd.dma_start(wq_t[:], wq_d.rearrange("(c p) m -> p c m", p=DC))
            wk_t = wp.tile([DC, NDC, WBLK], f32r)
            nc.gpsimd.dma_start(wk_t[:], wk_d.rearrange("(c p) m -> p c m", p=DC))
            wv_t = wp.tile([DC, NDC, WBLK], f32r)
            nc.gpsimd.dma_start(wv_t[:], wv_d.rearrange("(c p) m -> p c m", p=DC))
            wo_t = wp.tile([WBLK, D], f32r)
            nc.gpsimd.dma_start(wo_t[:], wo_d[:, :])

            for b in range(B):
                # ---------------- phase A: QKV projection ----------------
                qt2 = qkp.tile([128, T], f32r, tag="qt2", name=f"qt2_b{b}")
                kt2 = qkp.tile([128, T], f32r, tag="kt2", name=f"kt2_b{b}")
                # vaug[:, kt, 0:65] = [V_h0 | 1]; [:, kt, 65:130] = [V_h1 | 1]
                vaug = vap.tile([128, NKT, 130], f32r, tag="vaug", name=f"vaug_b{b}")
                nc.gpsimd.dma_start(
                    vaug[:, :, 64:130:65],
                    ones_d[:, 0:2 * NKT].rearrange("p (a b) -> p a b", b=2),
                )

                for tcq in range(NQC):
                    accq = mixp.tile([128, QC], f32, tag="mix", name=f"accq_b{b}t{tcq}")
                    acck = mixp.tile([128, QC], f32, tag="mix", name=f"acck_b{b}t{tcq}")
                    accv = mixp.tile([128, QC], f32, tag="mix", name=f"accv_b{b}t{tcq}")
                    xts = []
                    for dc in range(NDC):
                        xt_t = xtp.tile([DC, QC], f32r, tag="xt", name=f"xt_b{b}t{tcq}d{dc}")
                        nc.gpsimd.dma_start(
                            xt_t[:],
                            xt_d[dc * DC:(dc + 1) * DC,
                                 b * T + tcq * QC: b * T + (tcq + 1) * QC],
                        )
                        xts.append(xt_t)
                    for dc in range(NDC):
                        st, sp = (dc == 0), (dc == NDC - 1)
                        nc.tensor.matmul(
                            accq[:], wq_t[:, dc, :], xts[dc][:],
                            start=st, stop=sp, skip_group_check=True,
                        )
                        nc.tensor.matmul(
                            acck[:], wk_t[:, dc, :], xts[dc][:],
                            start=st, stop=sp, skip_group_check=True,
                        )
                    # fp32r accumulation groups interleaved within one psum
                    # tile corrupt results - keep each region's 8 matmuls
                    # contiguous (ttl outer, dc inner).
                    for ttl in range(QC // KT):
                        for dc in range(NDC):
                            nc.tensor.matmul(
                                accv[:, ttl * 128:(ttl + 1) * 128],
                                xts[dc][:, ttl * 128:(ttl + 1) * 128],
                                wv_t[:, dc, :],
                                start=(dc == 0), stop=(dc == NDC - 1),
                                skip_group_check=True,
                            )
                    nc.vector.tensor_copy(qt2[:, tcq * QC:(tcq + 1) * QC], accq[:])
                    nc.vector.tensor_copy(kt2[:, tcq * QC:(tcq + 1) * QC], acck[:])
                    for ttl in range(QC // KT):
                        kt_g = tcq * (QC // KT) + ttl
                        nc.vector.tensor_copy(
                            vaug[:, kt_g, 0:64], accv[:, ttl * 128: ttl * 128 + 64]
                        )
                        nc.vector.tensor_copy(
                            vaug[:, kt_g, 65:129], accv[:, ttl * 128 + 64: ttl * 128 + 128]
                        )

                # ------------- phase B + C: attention per query chunk -------------
                for qc in range(NQC):
                    nkt = (qc + 1) * (QC // KT)
                    qsl = slice(qc * QC, (qc + 1) * QC)
                    ot0 = otp.tile([65, QC], f32, tag="ot0", name=f"ot0_b{b}q{qc}")
                    ot1 = otp.tile([65, QC], f32, tag="ot1", name=f"ot1_b{b}q{qc}")
                    pend = None  # (expst0, expst1, kt) awaiting AV matmul
                    for kt in range(nkt):
                        ksl = slice(kt * KT, (kt + 1) * KT)
                        st0 = stp.tile([128, QC], f32, tag="st", name=f"st0_b{b}q{qc}k{kt}")
                        nc.tensor.matmul(
                            st0[:], kt2[0:64, ksl], qt2[0:64, qsl],
                            start=True, stop=True, skip_group_check=True,
                        )
                        st1 = stp.tile([128, QC], f32, tag="st", name=f"st1_b{b}q{qc}k{kt}")
                        nc.tensor.matmul(
                            st1[:], kt2[64:128, ksl], qt2[64:128, qsl],
                            start=True, stop=True, skip_group_check=True,
                        )
                        j = kt - qc * (QC // KT)
                        if 0 <= j < 4:
                            nc.vector.tensor_add(st0[:], st0[:], maskadd[j][:])
                            nc.vector.tensor_add(st1[:], st1[:], maskadd[j][:])
                        e0 = expp.tile([128, QC], f32r, tag="e0", name=f"e0_b{b}q{qc}k{kt}")
                        nc.scalar.activation(e0[:], st0[:], Exp, scale=0.125)
                        e1 = expp.tile([128, QC], f32r, tag="e1", name=f"e1_b{b}q{qc}k{kt}")
                        nc.scalar.activation(e1[:], st1[:], Exp, scale=0.125)
                        if pend is not None:
                            p0, p1, pk = pend
                            nc.tensor.matmul(
                                ot0[:], vaug[:, pk, 0:65], p0[:],
                                start=(pk == 0), stop=False, skip_group_check=True,
                            )
                            nc.tensor.matmul(
                                ot1[:], vaug[:, pk, 65:130], p1[:],
                                start=(pk == 0), stop=False, skip_group_check=True,
                            )
                        pend = (e0, e1, kt)
                    p0, p1, pk = pend
                    nc.tensor.matmul(
                        ot0[:], vaug[:, pk, 0:65], p0[:],
                        start=(pk == 0), stop=True, skip_group_check=True,
                    )
                    nc.tensor.matmul(
                        ot1[:], vaug[:, pk, 65:130], p1[:],
                        start=(pk == 0), stop=True, skip_group_check=True,
                    )

                    # normalize: otn[h*64:(h+1)*64, :] = ot_h[0:64] * (1/denom_h)
                    otn = otnp.tile([128, QC], f32r, tag="otn", name=f"otn_b{b}q{qc}")
                    for h, ot in ((0, ot0), (1, ot1)):
                        rcp = rcpp.tile([1, QC], f32r, tag=f"r{h}", name=f"rcp_b{b}q{qc}h{h}")
                        with nc.allow_low_precision(reason="f32r recip feeding bcast matmul"):
                            nc.vector.reciprocal(rcp[:], ot[64:65, :])
                        bc = mixp.tile([64, QC], f32, tag="mix", name=f"bc_b{b}q{qc}h{h}")
                        nc.tensor.matmul(
                            bc[:], ones64[:], rcp[:],
                            start=True, stop=True, skip_group_check=True,
                        )
                        bcs = bcp.tile([64, QC], f32, tag="bcs", name=f"bcs_b{b}q{qc}h{h}")
                        nc.vector.tensor_copy(bcs[:], bc[:])
                        nc.vector.tensor_mul(
                            otn[h * 64:(h + 1) * 64, :], ot[0:64, :], bcs[:]
                        )

                    # out projection: out[t, :] += OTn.T @ W_out2h
                    for ts in range(QC // KT):
                        row0 = b * T + qc * QC + ts * 128
                        for nn2 in range(D // QC):
                            ops = mixp.tile(
                                [128, QC], f32, tag="mix", name=f"ops_b{b}q{qc}s{ts}n{nn2}"
                            )
                            nc.tensor.matmul(
                                ops[:],
                                otn[:, ts * 128:(ts + 1) * 128],
                                wo_t[:, nn2 * QC:(nn2 + 1) * QC],
                                start=True, stop=True, skip_group_check=True,
                            )
                            osb = outsbp.tile(
                                [128, QC], f32, tag="osb", name=f"osb_b{b}q{qc}s{ts}n{nn2}"
                            )
                            nc.vector.tensor_copy(osb[:], ops[:])
                            nc.sync.dma_start(
                                out_d[row0:row0 + 128, nn2 * QC:(nn2 + 1) * QC], osb[:]
                            )
    return nc


def kernel(x, W_qkv, W_out):
    global LAST_EXEC_NS, LAST_RESULTS
    from concourse.bass_utils import run_bass_kernel_spmd

    if TRACE:
        _install_ntff_hook()

    x = np.ascontiguousarray(x, dtype=np.float32)
    W_qkv = np.ascontiguousarray(W_qkv, dtype=np.float32)
    W_out = np.ascontiguousarray(W_out, dtype=np.float32)

    xT = np.ascontiguousarray(x.transpose(2, 0, 1).reshape(D, B * T))
    in_maps = []
    for c in range(NCORES):
        cs = slice(c * WBLK, (c + 1) * WBLK)
        in_maps.append({
            "xt": xT,
            "wq": np.ascontiguousarray(W_qkv[:, 0 * D:1 * D][:, cs]),
            "wk": np.ascontiguousarray(W_qkv[:, 1 * D:2 * D][:, cs]),
            "wv": np.ascontiguousarray(W_qkv[:, 2 * D:3 * D][:, cs]),
            "wo": np.ascontiguousarray(W_out[cs, :]),
            "ones": np.ones((128, 64), dtype=np.float32),
        })

    nc = _build_program()
    res = run_bass_kernel_spmd(nc, in_maps, list(range(NCORES)), trace=TRACE)
    LAST_EXEC_NS = res.exec_time_ns
    LAST_RESULTS = res

    out = np.zeros((B * T, D), dtype=np.float64)
    for c in range(NCORES):
        out += res.results[c]["out"].astype(np.float64)
    return out.astype(np.float32).reshape(B, T, D)


# revision 17
# speedup vs baseline: 1.0391x; 1.0391x over previous
"""Causal multi-head attention (B=4, T=2048, D=1024, H=16) on 8 TRN2 NeuronCores.

Sharding: tensor-parallel over heads. Each core owns 2 heads (a contiguous
128-column block of each of W_q / W_k / W_v and a 128-row block of W_out).
Every core consumes the full (transposed) activation matrix xT and produces a
partial output [B*T, D]; the host sums the 8 partials (the "all-reduce").

Per-core device pipeline (all matmuls in float32r — full PE rate, ~1e-4 rel):
  phase A (per batch b): QT2[128,T] = Wq2h.T @ xT_b, KT2 likewise,
           V2[T,128] = xT_b.T @ Wv2h  (stored as Vaug tiles [128,(64|1)x2]
           with a ones column appended per head).
  phase B (per b, per 512-wide query chunk qc): for each 128-wide key tile kt:
           ST_h[k,q] = KT2_h.T @ QT2_h  (scores, transposed layout),
           causal mask via additive -1e30 tile on the diagonal blocks,
           expST = exp(ST/8) on ACT (straight-through softmax, no max-sub),
           OT_h[65,512] += Vaug_h.T @ expST  (row 64 accumulates the softmax
           denominator via the ones column).
  phase C (per b, qc): recip = 1/OT[64,:], broadcast across partitions with a
           rank-1 PE matmul, OTn = OT[0:64]*recip (both heads -> [128,512]),
           out[t,:] partial = OTn.T @ W_out2h, DMA to DRAM.
"""

import sys
import os

if "/opt/trn_rl_repo" not in sys.path:
    sys.path.insert(0, "/opt/trn_rl_repo")

import numpy as np

B, T, D, H, HD = 4, 2048, 1024, 16, 64
NCORES = 8
HPC = H // NCORES          # heads per core = 2
WBLK = HPC * HD            # 128: per-core head-block width
QC = 512                   # query chunk (matmul moving dim)
NQC = T // QC              # 4
KT = 128                   # key tile
NKT = T // KT              # 16
DC = 128                   # contraction chunk of D
NDC = D // DC              # 8

TRACE = False              # test.py sets kernel.TRACE = True for profiling
LAST_EXEC_NS = None
LAST_RESULTS = None

_MAX_WAITS = 1


def _make_tc_class():
    """TileContext patched for this container's walrus build, which rejects
    instructions carrying more than one sync-wait command (CTRL Drain,
    S3_LW ldweights, ...). Excess waits are hoisted onto freshly inserted
    same-engine NOPs placed immediately before the instruction (engine
    queues are in-order, so semantics are preserved)."""
    import concourse.tile as tile
    import concourse.mybir as mybir
    from concourse.vector_clock import VectorClock, ScopedClock

    class TC(tile.TileContext):
        def _drain_and_barrier(self, tick_clock, wait_clock):
            g = tick_clock.global_clock
            n = len(g)
            for proc in range(n):
                t = g[proc]
                if t > 0:
                    nop = self.nc.sync.nop(nofuse=True)
                    vc = VectorClock([0] * n)
                    vc.require_at_least(proc, t)
                    wait_clock.add_sem_waits(nop.ins, ScopedClock({None: vc}))
            self.nc.sync.drain()
            self.nc.all_engine_barrier()
            popped = self.nc._tile_sem_poison_stack.pop()
            assert popped is self._sem_poison
            self.nc.clear_and_free_semaphores(list(self.sems.allocated().values()))
            self.nc.all_engine_barrier()

        def _lower_ordered_insts(self, ordered):
            for bb_name in list(ordered.keys()):
                insts = ordered[bb_name]
                new_insts = []
                for inst in insts:
                    si = inst.sync_info
                    ow = list(si.on_wait) if si is not None and si.on_wait else []
                    if len(ow) > _MAX_WAITS:
                        keep = ow[:_MAX_WAITS]
                        extra = ow[_MAX_WAITS:]
                        for w in extra:
                            nop = mybir.InstNoOp(
                                name=f"WSPL-{self.nc.next_id()}", ins=[], outs=[]
                            )
                            nop.engine = inst.engine
                            nop.bass_nofuse = True
                            nop.sync_info = mybir.SyncInfo(on_wait=[w], on_update=[])
                            new_insts.append(nop)
                        inst.sync_info = mybir.SyncInfo(
                            on_wait=keep,
                            on_update=list(si.on_update) if si.on_update else [],
                        )
                    new_insts.append(inst)
                ordered[bb_name] = new_insts
            return super()._lower_ordered_insts(ordered)

    return TC


def _install_ntff_hook():
    """Provide antenv.axon_hooks (absent from the container's antenv stub) so
    run_bass_kernel_spmd(trace=True) can capture NTFF profiles."""
    import types
    import antenv

    if "antenv.axon_hooks" in sys.modules:
        return
    mod = types.ModuleType("antenv.axon_hooks")
    mod._hook = None
    mod.set_axon_ntff_profile_hook = lambda h: setattr(mod, "_hook", h)
    mod.get_axon_ntff_profile_hook = lambda: mod._hook
    sys.modules["antenv.axon_hooks"] = mod
    antenv.axon_hooks = mod
    try:
        from trn_agent_boot.trn_boot import _ntff_profile_via_ctypes

        hook = _ntff_profile_via_ctypes("/opt/axon/libaxon_pjrt.so")
        if hook is not None:
            mod.set_axon_ntff_profile_hook(hook)
    except Exception as e:  # profiling is best-effort
        print("ntff hook install failed:", e)


def _build_program():
    import concourse.bass as bass
    from concourse import mybir

    TC = _make_tc_class()
    f32 = mybir.dt.float32
    f32r = mybir.dt.float32r
    Exp = mybir.ActivationFunctionType.Exp
    BT = B * T

    nc = bass.Bass("TRN2", target_bir_lowering=False, debug=False, num_devices=NCORES)
    xt_d = nc.dram_tensor("xt", [D, BT], f32, kind="ExternalInput")
    wq_d = nc.dram_tensor("wq", [D, WBLK], f32, kind="ExternalInput")
    wk_d = nc.dram_tensor("wk", [D, WBLK], f32, kind="ExternalInput")
    wv_d = nc.dram_tensor("wv", [D, WBLK], f32, kind="ExternalInput")
    wo_d = nc.dram_tensor("wo", [WBLK, D], f32, kind="ExternalInput")
    ones_d = nc.dram_tensor("ones", [128, 64], f32, kind="ExternalInput")
    out_d = nc.dram_tensor("out", [BT, D], f32, kind="ExternalOutput")

    with TC(nc, num_cores=NCORES) as tc:
        from contextlib import ExitStack

        with ExitStack() as ctx:
            constp = ctx.enter_context(tc.tile_pool(name="constp", bufs=1))
            wp = ctx.enter_context(tc.tile_pool(name="wp", bufs=1))
            xtp = ctx.enter_context(tc.tile_pool(name="xtp", bufs=10))
            qkp = ctx.enter_context(tc.tile_pool(name="qkp", bufs=2))
            vap = ctx.enter_context(tc.tile_pool(name="vap", bufs=2))
            expp = ctx.enter_context(tc.tile_pool(name="expp", bufs=3))
            otnp = ctx.enter_context(tc.tile_pool(name="otnp", bufs=2))
            outsbp = ctx.enter_context(tc.tile_pool(name="outsbp", bufs=3))
            rcpp = ctx.enter_context(tc.tile_pool(name="rcpp", bufs=2))
            bcp = ctx.enter_context(tc.tile_pool(name="bcp", bufs=2))
            # PSUM: st(3) + mix(3) + ot0(1) + ot1(1) = 8 banks
            stp = ctx.enter_context(tc.tile_pool(name="stp", bufs=3, space="PSUM"))
            mixp = ctx.enter_context(tc.tile_pool(name="mixp", bufs=3, space="PSUM"))
            otp = ctx.enter_context(tc.tile_pool(name="otp", bufs=1, space="PSUM"))

            # ---- one-time constants / weights ----
            ones64 = constp.tile([1, 64], f32r)
            nc.gpsimd.dma_start(ones64[:], ones_d[0:1, 0:64])
            ident = constp.tile([128, 128], f32)
            from concourse.masks import make_identity
            make_identity(nc, ident[:])
            maskadd = []
            for j in range(QC // KT):  # 4 diagonal offsets
                m = constp.tile([128, QC], f32, name=f"maskadd{j}")
                nc.gpsimd.memset(m[:], 0.0)
                # keep (add 0) where q - k >= 0: q = qc0 + y, k = qc0 + 128j + x
                nc.gpsimd.affine_select(
                    out=m[:],
                    in_=m[:],
                    compare_op=mybir.AluOpType.is_ge,
                    fill=-1e30,
                    base=-KT * j,
                    pattern=[[1, QC]],
                    channel_multiplier=-1,
                )
                maskadd.append(m)

            wq_t = wp.tile([DC, NDC, WBLK], f32r)
            nc.gpsimd.dma_start(wq_t[:], wq_d.rearrange("(c p) m -> p c m", p=DC))
            wk_t = wp.tile([DC, NDC, WBLK], f32r)
            nc.gpsimd.dma_start(wk_t[:], wk_d.rearrange("(c p) m -> p c m", p=DC))
            wv_t = wp.tile([DC, NDC, WBLK], f32r)
            nc.gpsimd.dma_start(wv_t[:], wv_d.rearrange("(c p) m -> p c m", p=DC))
            wo_t = wp.tile([WBLK, D], f32r)
            nc.gpsimd.dma_start(wo_t[:], wo_d[:, :])

            for b in range(B):
                # ---------------- phase A: QKV projection ----------------
                qt2 = qkp.tile([128, T], f32r, tag="qt2", name=f"qt2_b{b}")
                kt2 = qkp.tile([128, T], f32r, tag="kt2", name=f"kt2_b{b}")
                # vaug[:, kt, 0:65] = [V_h0 | 1]; [:, kt, 65:130] = [V_h1 | 1]
                vaug = vap.tile([128, NKT, 130], f32r, tag="vaug", name=f"vaug_b{b}")
                nc.gpsimd.dma_start(
                    vaug[:, :, 64:130:65],
                    ones_d[:, 0:2 * NKT].rearrange("p (a b) -> p a b", b=2),
                )

                for tcq in range(NQC):
                    accq = mixp.tile([128, QC], f32, tag="mix", name=f"accq_b{b}t{tcq}")
                    acck = mixp.tile([128, QC], f32, tag="mix", name=f"acck_b{b}t{tcq}")
                    accv = mixp.tile([128, QC], f32, tag="mix", name=f"accv_b{b}t{tcq}")
                    xts = []
                    for dc in range(NDC):
                        xt_t = xtp.tile([DC, QC], f32r, tag="xt", name=f"xt_b{b}t{tcq}d{dc}")
                        nc.gpsimd.dma_start(
                            xt_t[:],
                            xt_d[dc * DC:(dc + 1) * DC,
                                 b * T + tcq * QC: b * T + (tcq + 1) * QC],
                        )
                        xts.append(xt_t)
                    for dc in range(NDC):
                        st, sp = (dc == 0), (dc == NDC - 1)
                        nc.tensor.matmul(
                            accq[:], wq_t[:, dc, :], xts[dc][:],
                            start=st, stop=sp, skip_group_check=True,
                        )
                        nc.tensor.matmul(
                            acck[:], wk_t[:, dc, :], xts[dc][:],
                            start=st, stop=sp, skip_group_check=True,
                        )
                        # V^T [128(2h*64), t]: N=512 streams instead of 4x N=128
                        nc.tensor.matmul(
                            accv[:], wv_t[:, dc, :], xts[dc][:],
                            start=st, stop=sp, skip_group_check=True,
                        )
                    nc.vector.tensor_copy(qt2[:, tcq * QC:(tcq + 1) * QC], accq[:])
                    nc.vector.tensor_copy(kt2[:, tcq * QC:(tcq + 1) * QC], acck[:])
                    # transpose V^T -> V via PE, 128x128 blocks
                    vt_sb = bcp.tile([128, QC], f32, tag="vtsb", name=f"vtsb_b{b}t{tcq}")
                    nc.vector.tensor_copy(vt_sb[:], accv[:])
                    vtr = mixp.tile([128, QC], f32, tag="mix", name=f"vtr_b{b}t{tcq}")
                    for ttl in range(QC // KT):
                        nc.tensor.transpose(
                            vtr[:, ttl * 128:(ttl + 1) * 128],
                            vt_sb[:, ttl * 128:(ttl + 1) * 128],
                            ident[:],
                        )
                    for ttl in range(QC // KT):
                        kt_g = tcq * (QC // KT) + ttl
                        nc.vector.tensor_copy(
                            vaug[:, kt_g, 0:64], vtr[:, ttl * 128: ttl * 128 + 64]
                        )
                        nc.vector.tensor_copy(
                            vaug[:, kt_g, 65:129], vtr[:, ttl * 128 + 64: ttl * 128 + 128]
                        )

                # ------------- phase B + C: attention per query chunk -------------
                for qc in range(NQC):
                    nkt = (qc + 1) * (QC // KT)
                    qsl = slice(qc * QC, (qc + 1) * QC)
                    ot0 = otp.tile([65, QC], f32, tag="ot0", name=f"ot0_b{b}q{qc}")
                    ot1 = otp.tile([65, QC], f32, tag="ot1", name=f"ot1_b{b}q{qc}")
                    pend = None  # (expst0, expst1, kt) awaiting AV matmul
                    for kt in range(nkt):
                        ksl = slice(kt * KT, (kt + 1) * KT)
                        st0 = stp.tile([128, QC], f32, tag="st", name=f"st0_b{b}q{qc}k{kt}")
                        nc.tensor.matmul(
                            st0[:], kt2[0:64, ksl], qt2[0:64, qsl],
                            start=True, stop=True, skip_group_check=True,
                        )
                        st1 = stp.tile([128, QC], f32, tag="st", name=f"st1_b{b}q{qc}k{kt}")
                        nc.tensor.matmul(
                            st1[:], kt2[64:128, ksl], qt2[64:128, qsl],
                            start=True, stop=True, skip_group_check=True,
                        )
                        j = kt - qc * (QC // KT)
                        if 0 <= j < 4:
                            nc.vector.tensor_add(st0[:], st0[:], maskadd[j][:])
                            nc.vector.tensor_add(st1[:], st1[:], maskadd[j][:])
                        e0 = expp.tile([128, QC], f32r, tag="e0", name=f"e0_b{b}q{qc}k{kt}")
                        nc.scalar.activation(e0[:], st0[:], Exp, scale=0.125)
                        e1 = expp.tile([128, QC], f32r, tag="e1", name=f"e1_b{b}q{qc}k{kt}")
                        nc.scalar.activation(e1[:], st1[:], Exp, scale=0.125)
                        if pend is not None:
                            p0, p1, pk = pend
                            nc.tensor.matmul(
                                ot0[:], vaug[:, pk, 0:65], p0[:],
                                start=(pk == 0), stop=False, skip_group_check=True,
                            )
                            nc.tensor.matmul(
                                ot1[:], vaug[:, pk, 65:130], p1[:],
                                start=(pk == 0), stop=False, skip_group_check=True,
                            )
                        pend = (e0, e1, kt)
                    p0, p1, pk = pend
                    nc.tensor.matmul(
                        ot0[:], vaug[:, pk, 0:65], p0[:],
                        start=(pk == 0), stop=True, skip_group_check=True,
                    )
                    nc.tensor.matmul(
                        ot1[:], vaug[:, pk, 65:130], p1[:],
                        start=(pk == 0), stop=True, skip_group_check=True,
                    )

                    # normalize: otn[h*64:(h+1)*64, :] = ot_h[0:64] * (1/denom_h)
                    # (broadcast the raw denominator row across 64 partitions
                    # with a rank-1 PE matmul first, THEN reciprocal on 64
                    # lanes - a [1,512] DVE reciprocal is single-lane, 3.3us)
                    otn = otnp.tile([128, QC], f32r, tag="otn", name=f"otn_b{b}q{qc}")
                    for h, ot in ((0, ot0), (1, ot1)):
                        den = rcpp.tile([1, QC], f32r, tag=f"r{h}", name=f"den_b{b}q{qc}h{h}")
                        nc.vector.tensor_copy(den[:], ot[64:65, :])
                        bc = mixp.tile([64, QC], f32, tag="mix", name=f"bc_b{b}q{qc}h{h}")
                        nc.tensor.matmul(
                            bc[:], ones64[:], den[:],
                            start=True, stop=True, skip_group_check=True,
                        )
                        bcs = bcp.tile([64, QC], f32, tag="bcs", name=f"bcs_b{b}q{qc}h{h}")
                        nc.vector.reciprocal(bcs[:], bc[:])
                        nc.vector.tensor_mul(
                            otn[h * 64:(h + 1) * 64, :], ot[0:64, :], bcs[:]
                        )

                    # out projection: out[t, :] += OTn.T @ W_out2h
                    for ts in range(QC // KT):
                        row0 = b * T + qc * QC + ts * 128
                        for nn2 in range(D // QC):
                            ops = mixp.tile(
                                [128, QC], f32, tag="mix", name=f"ops_b{b}q{qc}s{ts}n{nn2}"
                            )
                            nc.tensor.matmul(
                                ops[:],
                                otn[:, ts * 128:(ts + 1) * 128],
                                wo_t[:, nn2 * QC:(nn2 + 1) * QC],
                                start=True, stop=True, skip_group_check=True,
                            )
                            osb = outsbp.tile(
                                [128, QC], f32, tag="osb", name=f"osb_b{b}q{qc}s{ts}n{nn2}"
                            )
                            nc.vector.tensor_copy(osb[:], ops[:])
                            nc.sync.dma_start(
                                out_d[row0:row0 + 128, nn2 * QC:(nn2 + 1) * QC], osb[:]
                            )
    return nc


def kernel(x, W_qkv, W_out):
    global LAST_EXEC_NS, LAST_RESULTS
    from concourse.bass_utils import run_bass_kernel_spmd

    if TRACE:
        _install_ntff_hook()

    x = np.ascontiguousarray(x, dtype=np.float32)
    W_qkv = np.ascontiguousarray(W_qkv, dtype=np.float32)
    W_out = np.ascontiguousarray(W_out, dtype=np.float32)

    xT = np.ascontiguousarray(x.transpose(2, 0, 1).reshape(D, B * T))
    in_maps = []
    for c in range(NCORES):
        cs = slice(c * WBLK, (c + 1) * WBLK)
        in_maps.append({
            "xt": xT,
            "wq": np.ascontiguousarray(W_qkv[:, 0 * D:1 * D][:, cs]),
            "wk": np.ascontiguousarray(W_qkv[:, 1 * D:2 * D][:, cs]),
            "wv": np.ascontiguousarray(W_qkv[:, 2 * D:3 * D][:, cs]),
            "wo": np.ascontiguousarray(W_out[cs, :]),
            "ones": np.ones((128, 64), dtype=np.float32),
        })

    nc = _build_program()
    res = run_bass_kernel_spmd(nc, in_maps, list(range(NCORES)), trace=TRACE)
    LAST_EXEC_NS = res.exec_time_ns
    LAST_RESULTS = res

    out = np.zeros((B * T, D), dtype=np.float64)
    for c in range(NCORES):
        out += res.results[c]["out"].astype(np.float64)
    return out.astype(np.float32).reshape(B, T, D)
